# revision 2
# baseline (speedup 1.0000x reference)
"""CondMlp Trainium2 kernel (v2).

Math (reference):
    xp = x @ W_pre + b_pre                 # [B, NI, DH]
    c  = query @ W_emb + b_emb             # [B, NO, DH]
    A  = xp @ W1[:DH] + b1                 # [B, NI, DH]   (host precompute, tiny)
    C2 = c @ W1[DH:]                       # [B, NO, DH]   (host precompute, tiny)
    h[b,i,o,:] = A[b,i,:] + C2[b,o,:]
    out[b,i,o,:] = gelu(h) @ W2 + b2       # [B, NI, NO, DOUT]

Sharding: 8 cores, core k handles batch b = k//2, NI-half h = k%2 (128 rows).

v2 design (vs the v1 baseline at ~119us):
  * bf16 output stores: halves the 32 MiB/core store traffic. rel-err budget
    (2e-2) dwarfs the ~0.2% bf16 rounding.
  * ACT does (almost) only gelu. v1 had ACT ~94us busy (gelu + half the
    drains); gelu alone has a hard 55us floor (1 elem/lane/cycle @1.2GHz),
    so every other byte moved through ACT was pure critical path.
  * Broadcast adds split between DVE and GPSIMD. The adds are 2x_1P on DVE
    (197ns per [128,256]); GPSIMD is otherwise idle and its tensor_scalar
    streams ~1-2.6 cyc/elem, so it absorbs a large fraction for free.
  * W2-stationary matmuls: out.T[dout, (rows,o)] = W2[dh,dout].T @ g.T[dh,*].
    The moving operand is the gelu output (already in [dh, (ch,r,o)] layout),
    N=512 per instr, and the stationary tile only changes 4x per 4-row
    subgroup (vs every matmul in v1). Output is transposed; host untransposes.
  * PSUM drains (fp32 -> bf16, the 1x port-bound wall) split DVE/ACT at
    FD=2048 to amortize fixed overheads.
  * A dozen dummy matmuls during the ramp warm the PE HAM clock-gate
    (1.2 -> 2.4 GHz) before the first real matmul.
"""

import numpy as np
import ml_dtypes

import concourse.bass as bass
import concourse.bacc as bacc
import concourse.mybir as mybir
from concourse.tile import TileContext
from concourse.bass_utils import run_bass_kernel_spmd

B, NI, NO = 4, 256, 256
DIN, DQ, DH, DOUT = 256, 256, 256, 256
NCORES = 8
RPC = (B * NI) // NCORES    # rows per core = 128
G16 = 16                    # rows per add/gelu group
NG = RPC // G16             # 8 groups
NSUB = RPC // 4             # 32 matmul subgroups of 4 rows
F32 = mybir.dt.float32
BF16 = mybir.dt.bfloat16

# Engine split knobs (tuned from traces).
GPS_NUM, GPS_DEN = 1, 2     # fraction of adds on GPSIMD
ACT_DRAIN_MOD = 4           # every k-th drain goes to ACT (rest DVE)

_nc_cache = None


def build_nc():
    nc = bacc.Bacc()

    c_t = nc.declare_dram_parameter("c_t", [DH, NO], BF16, isOutput=False)
    a_t = nc.declare_dram_parameter("a_t", [DH, RPC], F32, isOutput=False)
    w2 = nc.declare_dram_parameter("w2", [DH, DOUT], BF16, isOutput=False)
    # out[u, p, (s2, d, r, o)]: u = store unit (2 subgroups of 4 rows),
    # p = dout within chunk, free = s2*2048 + d*1024 + r*256 + o. Host
    # untransposes (the device computes out.T tiles).
    out = nc.declare_dram_parameter("out", [NSUB // 2, 128, 4096], BF16,
                                    isOutput=True)

    gelu = mybir.ActivationFunctionType.Gelu

    with TileContext(nc) as tc:
        with (
            tc.tile_pool(name="const", bufs=1) as cpool,
            tc.tile_pool(name="h", bufs=2) as hpool,
            tc.tile_pool(name="g", bufs=2) as gpool,
            tc.tile_pool(name="ps", bufs=2, space="PSUM") as pspool,
            tc.tile_pool(name="ostage", bufs=3) as opool,
        ):
            # Constant loads, all on the sync HWDGE ring (gpsimd does compute
            # now, so its SWDGE ring stays out of the picture).
            ct = []
            at = []
            w2t = []
            for ch in range(2):
                t = cpool.tile([128, NO], BF16, tag=f"ct{ch}")
                nc.sync.dma_start(out=t[:, :], in_=c_t[ch * 128:(ch + 1) * 128, :])
                ct.append(t)
                t = cpool.tile([128, RPC], F32, tag=f"at{ch}")
                nc.sync.dma_start(out=t[:, :], in_=a_t[ch * 128:(ch + 1) * 128, :])
                at.append(t)
                t = cpool.tile([128, DOUT], BF16, tag=f"w2{ch}")
                nc.sync.dma_start(out=t[:, :], in_=w2[ch * 128:(ch + 1) * 128, :])
                w2t.append(t)

            # ACT warmup: pays the ~2.7us gelu table load during the ramp.
            scratch = cpool.tile([128, 2], F32, tag="scratch")
            nc.vector.memset(scratch[:, :], 0.0)
            nc.scalar.activation(scratch[:, :], scratch[:, :], gelu)

            # PE warmup: ~3.5us of dummy matmuls flips the HAM clock-gate to
            # 8/8 (2.4 GHz) before the first real matmul (PE is otherwise
            # idle until ~4.5us while adds/gelu fill the pipeline).
            dummy = cpool.tile([128, 128], BF16, tag="dummy")
            nc.vector.memset(dummy[:, :], 0.0)
            ps_w = pspool.tile([128, 2048], F32, tag="ps")
            for i in range(12):
                nc.tensor.matmul(out=ps_w[:, 0:128], lhsT=dummy[:, :],
                                 rhs=dummy[:, :], start=True, stop=True)

            add_idx = 0
            drain_idx = 0
            for g in range(NG):
                # h/g free layout: (ch, r, o) -> ch*4096 + r*256 + o
                h_buf = hpool.tile([128, G16 * 512], BF16, tag="h")
                g_buf = gpool.tile([128, G16 * 512], BF16, tag="g")

                for ch in range(2):
                    for r in range(G16):
                        row = g * G16 + r
                        s = ch * 4096 + r * 256
                        eng = (nc.gpsimd
                               if (add_idx * GPS_NUM) % GPS_DEN < GPS_NUM
                               else nc.vector)
                        eng.tensor_scalar_add(
                            out=h_buf[:, s:s + 256],
                            in0=ct[ch][:, :],
                            scalar1=at[ch][:, row:row + 1],
                        )
                        add_idx += 1
                    nc.scalar.activation(
                        g_buf[:, ch * 4096:(ch + 1) * 4096],
                        h_buf[:, ch * 4096:(ch + 1) * 4096], gelu)

                # 4 subgroups of 4 rows; 2 subgroups share one 1 MiB store.
                for s2 in range(2):
                    ost = opool.tile([128, 4096], BF16, tag="ostage")
                    for s4i in range(2):
                        s4 = s2 * 2 + s4i
                        ps = pspool.tile([128, 2048], F32, tag="ps")
                        # ps free layout: (d, rpair, o) -> d*1024 + j*512 + ...
                        for d in range(2):
                            for ch in range(2):
                                for j in range(2):
                                    r0 = s4 * 4 + 2 * j
                                    nc.tensor.matmul(
                                        out=ps[:, d * 1024 + j * 512:
                                               d * 1024 + (j + 1) * 512],
                                        lhsT=w2t[ch][:, d * 128:(d + 1) * 128],
                                        rhs=g_buf[:, ch * 4096 + r0 * 256:
                                                  ch * 4096 + r0 * 256 + 512],
                                        start=(ch == 0), stop=(ch == 1),
                                    )
                        dst = ost[:, s4i * 2048:(s4i + 1) * 2048]
                        if drain_idx % ACT_DRAIN_MOD == 1:
                            nc.scalar.copy(dst, ps[:, :])
                        else:
                            nc.vector.tensor_copy(dst, ps[:, :])
                        drain_idx += 1
                    u = g * 2 + s2
                    nc.sync.dma_start(out=out[u], in_=ost[:, :])

    nc.finalize()
    return nc


def _get_nc():
    global _nc_cache
    if _nc_cache is None:
        _nc_cache = build_nc()
    return _nc_cache


def make_in_maps(x, query, W_pre, b_pre, W_emb, b_emb, W1, b1, W2, b2):
    x = np.asarray(x, np.float32)
    query = np.asarray(query, np.float32)
    W_pre = np.asarray(W_pre, np.float32)
    b_pre = np.asarray(b_pre, np.float32)
    W_emb = np.asarray(W_emb, np.float32)
    b_emb = np.asarray(b_emb, np.float32)
    W1 = np.asarray(W1, np.float32)
    b1 = np.asarray(b1, np.float32)
    W2 = np.asarray(W2, np.float32)

    xp = x.reshape(B * NI, DIN) @ W_pre + b_pre
    A = xp @ W1[:DH] + b1                       # [B*NI, DH]
    c = query.reshape(B * NO, DQ) @ W_emb + b_emb
    C2 = c @ W1[DH:]                            # [B*NO, DH]
    A = A.reshape(B, NI, DH)
    C2 = C2.reshape(B, NO, DH)

    w2b = np.ascontiguousarray(W2.astype(ml_dtypes.bfloat16))
    in_maps = []
    for k in range(NCORES):
        b = k // 2
        hh = k % 2
        in_maps.append({
            "c_t": np.ascontiguousarray(C2[b].T.astype(ml_dtypes.bfloat16)),
            "a_t": np.ascontiguousarray(A[b, hh * 128:(hh + 1) * 128, :].T),
            "w2": w2b,
        })
    return in_maps


def run_on_device(in_maps, trace=False):
    nc = _get_nc()
    return run_bass_kernel_spmd(nc, in_maps, core_ids=list(range(NCORES)), trace=trace)


def assemble(results, b2):
    out = np.empty((B, NI, NO, DOUT), np.float32)
    for k in range(NCORES):
        b = k // 2
        hh = k % 2
        # dev out: [u, p, (s2, d, r, o)] -> out[b, (u*2+s2)*4+r, o, d*128+p]
        dev = results[k]["out"].reshape(NSUB // 2, 128, 2, 2, 4, 256)
        # axes (u, p, s2, d, r, o) -> (u, s2, r, o, d, p)
        dev = dev.transpose(0, 2, 4, 5, 3, 1).reshape(RPC, NO, DOUT)
        out[b, hh * 128:(hh + 1) * 128] = dev.astype(np.float32)
    b2 = np.asarray(b2, np.float32)
    if np.any(b2):
        out += b2
    return out


def kernel(x, query, W_pre, b_pre, W_emb, b_emb, W1, b1, W2, b2):
    in_maps = make_in_maps(x, query, W_pre, b_pre, W_emb, b_emb, W1, b1, W2, b2)
    res = run_on_device(in_maps, trace=False)
    return assemble(res.results, b2)


# revision 4
# speedup vs baseline: 4.5422x; 4.5422x over previous
"""CondMlp Trainium2 kernel (v2).

Math (reference):
    xp = x @ W_pre + b_pre                 # [B, NI, DH]
    c  = query @ W_emb + b_emb             # [B, NO, DH]
    A  = xp @ W1[:DH] + b1                 # [B, NI, DH]   (host precompute, tiny)
    C2 = c @ W1[DH:]                       # [B, NO, DH]   (host precompute, tiny)
    h[b,i,o,:] = A[b,i,:] + C2[b,o,:]
    out[b,i,o,:] = gelu(h) @ W2 + b2       # [B, NI, NO, DOUT]

Sharding: 8 cores, core k handles batch b = k//2, NI-half h = k%2 (128 rows).

v2 design (vs the v1 baseline at ~119us):
  * bf16 output stores: halves the 32 MiB/core store traffic. rel-err budget
    (2e-2) dwarfs the ~0.2% bf16 rounding.
  * ACT does (almost) only gelu. v1 had ACT ~94us busy (gelu + half the
    drains); gelu alone has a hard 55us floor (1 elem/lane/cycle @1.2GHz),
    so every other byte moved through ACT was pure critical path.
  * Broadcast adds split between DVE and GPSIMD. The adds are 2x_1P on DVE
    (197ns per [128,256]); GPSIMD is otherwise idle and its tensor_scalar
    streams ~1-2.6 cyc/elem, so it absorbs a large fraction for free.
  * W2-stationary matmuls: out.T[dout, (rows,o)] = W2[dh,dout].T @ g.T[dh,*].
    The moving operand is the gelu output (already in [dh, (ch,r,o)] layout),
    N=512 per instr, and the stationary tile only changes 4x per 4-row
    subgroup (vs every matmul in v1). Output is transposed; host untransposes.
  * PSUM drains (fp32 -> bf16, the 1x port-bound wall) split DVE/ACT at
    FD=2048 to amortize fixed overheads.
  * A dozen dummy matmuls during the ramp warm the PE HAM clock-gate
    (1.2 -> 2.4 GHz) before the first real matmul.
"""

import numpy as np
import ml_dtypes

import concourse.bass as bass
import concourse.bacc as bacc
import concourse.mybir as mybir
from concourse.tile import TileContext
from concourse.bass_utils import run_bass_kernel_spmd

B, NI, NO = 4, 256, 256
DIN, DQ, DH, DOUT = 256, 256, 256, 256
NCORES = 8
RPC = (B * NI) // NCORES    # rows per core = 128
G16 = 16                    # rows per add/gelu group
NG = RPC // G16             # 8 groups
NSUB = RPC // 4             # 32 matmul subgroups of 4 rows
F32 = mybir.dt.float32
BF16 = mybir.dt.bfloat16

# Engine split knobs (tuned from traces). GPSIMD's stock tensor_scalar ucode
# measured ~3.9us per [128,256] add (20x the DVE) — never route adds there.
ACT_DRAIN_MOD = 2           # every k-th drain goes to ACT (rest DVE)

_nc_cache = None


def build_nc():
    nc = bacc.Bacc()

    c_t = nc.declare_dram_parameter("c_t", [DH, NO], BF16, isOutput=False)
    a_t = nc.declare_dram_parameter("a_t", [DH, RPC], F32, isOutput=False)
    w2 = nc.declare_dram_parameter("w2", [DH, DOUT], BF16, isOutput=False)
    # out[u, p, (s2, d, r, o)]: u = store unit (2 subgroups of 4 rows),
    # p = dout within chunk, free = s2*2048 + d*1024 + r*256 + o. Host
    # untransposes (the device computes out.T tiles).
    out = nc.declare_dram_parameter("out", [NSUB // 2, 128, 4096], BF16,
                                    isOutput=True)

    gelu = mybir.ActivationFunctionType.Gelu

    with TileContext(nc) as tc:
        with (
            tc.tile_pool(name="const", bufs=1) as cpool,
            tc.tile_pool(name="h", bufs=2) as hpool,
            tc.tile_pool(name="g", bufs=2) as gpool,
            tc.tile_pool(name="ps", bufs=2, space="PSUM") as pspool,
            tc.tile_pool(name="ostage", bufs=3) as opool,
        ):
            # Constant loads, all on the sync HWDGE ring (gpsimd does compute
            # now, so its SWDGE ring stays out of the picture).
            ct = []
            at = []
            w2t = []
            for ch in range(2):
                t = cpool.tile([128, NO], BF16, tag=f"ct{ch}")
                nc.sync.dma_start(out=t[:, :], in_=c_t[ch * 128:(ch + 1) * 128, :])
                ct.append(t)
                t = cpool.tile([128, RPC], F32, tag=f"at{ch}")
                nc.sync.dma_start(out=t[:, :], in_=a_t[ch * 128:(ch + 1) * 128, :])
                at.append(t)
                t = cpool.tile([128, DOUT], BF16, tag=f"w2{ch}")
                nc.sync.dma_start(out=t[:, :], in_=w2[ch * 128:(ch + 1) * 128, :])
                w2t.append(t)

            # ACT warmup: pays the ~2.7us gelu table load during the ramp.
            scratch = cpool.tile([128, 2], F32, tag="scratch")
            nc.vector.memset(scratch[:, :], 0.0)
            nc.scalar.activation(scratch[:, :], scratch[:, :], gelu)

            # PE warmup: ~3.5us of dummy matmuls flips the HAM clock-gate to
            # 8/8 (2.4 GHz) before the first real matmul (PE is otherwise
            # idle until ~4.5us while adds/gelu fill the pipeline).
            dummy = cpool.tile([128, 128], BF16, tag="dummy")
            nc.vector.memset(dummy[:, :], 0.0)
            ps_w = pspool.tile([128, 2048], F32, tag="ps")
            for i in range(12):
                nc.tensor.matmul(out=ps_w[:, 0:128], lhsT=dummy[:, :],
                                 rhs=dummy[:, :], start=True, stop=True)

            add_idx = 0
            drain_idx = 0
            for g in range(NG):
                # h/g free layout: (ch, r, o) -> ch*4096 + r*256 + o
                h_buf = hpool.tile([128, G16 * 512], BF16, tag="h")
                g_buf = gpool.tile([128, G16 * 512], BF16, tag="g")

                for ch in range(2):
                    for r in range(G16):
                        row = g * G16 + r
                        s = ch * 4096 + r * 256
                        nc.vector.tensor_scalar_add(
                            out=h_buf[:, s:s + 256],
                            in0=ct[ch][:, :],
                            scalar1=at[ch][:, row:row + 1],
                        )
                        add_idx += 1
                    nc.scalar.activation(
                        g_buf[:, ch * 4096:(ch + 1) * 4096],
                        h_buf[:, ch * 4096:(ch + 1) * 4096], gelu)

                # 4 subgroups of 4 rows; 2 subgroups share one 1 MiB store.
                for s2 in range(2):
                    ost = opool.tile([128, 4096], BF16, tag="ostage")
                    for s4i in range(2):
                        s4 = s2 * 2 + s4i
                        ps = pspool.tile([128, 2048], F32, tag="ps")
                        # ps free layout: (d, rpair, o) -> d*1024 + j*512 + ...
                        for d in range(2):
                            for ch in range(2):
                                for j in range(2):
                                    r0 = s4 * 4 + 2 * j
                                    nc.tensor.matmul(
                                        out=ps[:, d * 1024 + j * 512:
                                               d * 1024 + (j + 1) * 512],
                                        lhsT=w2t[ch][:, d * 128:(d + 1) * 128],
                                        rhs=g_buf[:, ch * 4096 + r0 * 256:
                                                  ch * 4096 + r0 * 256 + 512],
                                        start=(ch == 0), stop=(ch == 1),
                                    )
                        dst = ost[:, s4i * 2048:(s4i + 1) * 2048]
                        if drain_idx % ACT_DRAIN_MOD == 1:
                            nc.scalar.copy(dst, ps[:, :])
                        else:
                            nc.vector.tensor_copy(dst, ps[:, :])
                        drain_idx += 1
                    u = g * 2 + s2
                    nc.sync.dma_start(out=out[u], in_=ost[:, :])

    nc.finalize()
    return nc


def _get_nc():
    global _nc_cache
    if _nc_cache is None:
        _nc_cache = build_nc()
    return _nc_cache


def make_in_maps(x, query, W_pre, b_pre, W_emb, b_emb, W1, b1, W2, b2):
    x = np.asarray(x, np.float32)
    query = np.asarray(query, np.float32)
    W_pre = np.asarray(W_pre, np.float32)
    b_pre = np.asarray(b_pre, np.float32)
    W_emb = np.asarray(W_emb, np.float32)
    b_emb = np.asarray(b_emb, np.float32)
    W1 = np.asarray(W1, np.float32)
    b1 = np.asarray(b1, np.float32)
    W2 = np.asarray(W2, np.float32)

    xp = x.reshape(B * NI, DIN) @ W_pre + b_pre
    A = xp @ W1[:DH] + b1                       # [B*NI, DH]
    c = query.reshape(B * NO, DQ) @ W_emb + b_emb
    C2 = c @ W1[DH:]                            # [B*NO, DH]
    A = A.reshape(B, NI, DH)
    C2 = C2.reshape(B, NO, DH)

    w2b = np.ascontiguousarray(W2.astype(ml_dtypes.bfloat16))
    in_maps = []
    for k in range(NCORES):
        b = k // 2
        hh = k % 2
        in_maps.append({
            "c_t": np.ascontiguousarray(C2[b].T.astype(ml_dtypes.bfloat16)),
            "a_t": np.ascontiguousarray(A[b, hh * 128:(hh + 1) * 128, :].T),
            "w2": w2b,
        })
    return in_maps


def run_on_device(in_maps, trace=False):
    nc = _get_nc()
    return run_bass_kernel_spmd(nc, in_maps, core_ids=list(range(NCORES)), trace=trace)


def assemble(results, b2):
    out = np.empty((B, NI, NO, DOUT), np.float32)
    for k in range(NCORES):
        b = k // 2
        hh = k % 2
        # dev out: [u, p, (s2, d, r, o)] -> out[b, (u*2+s2)*4+r, o, d*128+p]
        dev = results[k]["out"].reshape(NSUB // 2, 128, 2, 2, 4, 256)
        # axes (u, p, s2, d, r, o) -> (u, s2, r, o, d, p)
        dev = dev.transpose(0, 2, 4, 5, 3, 1).reshape(RPC, NO, DOUT)
        out[b, hh * 128:(hh + 1) * 128] = dev.astype(np.float32)
    b2 = np.asarray(b2, np.float32)
    if np.any(b2):
        out += b2
    return out


def kernel(x, query, W_pre, b_pre, W_emb, b_emb, W1, b1, W2, b2):
    in_maps = make_in_maps(x, query, W_pre, b_pre, W_emb, b_emb, W1, b1, W2, b2)
    res = run_on_device(in_maps, trace=False)
    return assemble(res.results, b2)
